# revision 23
# baseline (speedup 1.0000x reference)
"""Causal multi-head self-attention on 8 Trainium2 NeuronCores.

Problem: x[2,2048,1024], 16 heads, dk=64, causal softmax, fp32 in/out.

Sharding: core c handles batch b = c//4 and head group g = c%4 (4 heads
= 2 pairs mi of 2). wq/wk/wv column-sharded, wo row-sharded; each core
returns a [D, S] fp16 partial of out^T for its batch; the host sums 4
partials per batch in fp64.

Precision plan (validated vs the fp32 reference, rel err 1.2e-2 < 2e-2;
errors at the attention core average out over ~1.7k effectively-attended
keys for this data):
  - q/k projections in fp8e4m3 DoubleRow (0.5 PE cycles/row, 2x
    contraction/instr): weights prescaled x64 on the host, descaled by
    1/64 in the psum->sbuf copy.
  - scores q@k in fp8 DoubleRow contracting dk=64 as [32 partitions x
    2 halves]; qT8/kT8 live in [32, head, dk-half, S] tiles produced
    from the projection psum via a DRAM round-trip DMA (the partition
    fold is impossible for lane-locked engines). q-chunks 0..1 (and
    the k tiles they need) instead use an fp16 path (qkT16) straight
    from psum, keeping the bounce latency off the critical path.
  - AV in fp8 DoubleRow over k-tile PAIRS for q >= 512 (e and v fp8;
    one exp per (head, jpair) over the union width; the fully-masked
    128-col hole of the upper k-tile is pre-filled with 16*-120=-1920
    by a K=1 matmul so exp lands exactly 0). v8 is projected from xT8
    in DoubleRow with a same-scale wv residual (v8 = x8@(wv8+dwv8)).
    q < 512 (few attended keys, no error averaging) stays fp16: v16
    from a [D, 512] fp16 x slice.
  - v parity layout per 128-wide head group (even: v@[0:64], ones col
    64; odd: v@[64:128], ones col 32) makes the AV matmul land each
    head's attn on the psum partitions wo wants, with the softmax
    denominator accumulated for free on rows 64/32.
  - causal diagonal via the v1 staircase (stA [t<=k] x stB -240*(t>q)).

Schedule: Act (exp) is the roofline (~8.5M exp elements/core = 58us
floor + per-instr overhead ~= 78us). Scores psum tiles [128, 2j, 512]
are double-buffered; each unit weaves the NEXT unit's first two score
tiles in before its own normalization so Act never waits at unit
boundaries. v/wo/projection work is emitted between attention steps as
PE filler through generator queues; v-tile generators drain lazily per
jpair. Unit order: (0,0) first (fp16, no bounce), then (0,1)..(1,3),
and the wo-chunk-3 tail runs across all free psum banks, all f=0
matmuls emitted back-to-back (continuous PE stream holds full pstate)
with ob copies alternating DVE / (then-idle) Act. DMA: bulk inputs ordered by
first use; bounce out-DMAs ride the Pool SWDGE queue, clear of the
serial HWDGE input stream.

TimelineSim: 116047 ns (v1 fp16 baseline: 159668 ns).
"""

import os
import sys

import numpy as np

if "/opt/trn_rl_repo" not in sys.path:
    sys.path.insert(0, "/opt/trn_rl_repo")

DEBUG = bool(os.environ.get("BASSDBG"))

B, S, D, H, DK = 2, 2048, 1024, 16, 64
HPC = 4            # heads per core
GW = HPC * DK      # 256
NCORES = 8
QC = 512           # q-chunk width
NQC = S // QC      # 4
KT = 128           # k-tile
KCP = 4            # 256-wide contraction pairs for DR projections
MASK_STEP = -240.0

_CACHE = {}


def _build_nc(reps=1):
    import concourse.bacc as bacc
    import concourse.tile as tile
    import concourse.bass as bass
    from concourse import mybir

    f32 = mybir.dt.float32
    fp16 = mybir.dt.float16
    fp8 = mybir.dt.float8e4
    Exp = mybir.ActivationFunctionType.Exp
    PSUM = bass.MemorySpace.PSUM
    DR = mybir.MatmulPerfMode.DoubleRow

    nc = bacc.Bacc(
        "TRN2",
        target_bir_lowering=False,
        debug=False,
        enable_asserts=False,
        num_devices=NCORES,
    )

    xT8_d = nc.dram_tensor("xT8", [D, S], fp8, kind="ExternalInput")
    xT16_d = nc.dram_tensor("xT16", [D, QC], fp16, kind="ExternalInput")
    wqk8_d = nc.dram_tensor("wqk8", [D, 2 * GW], fp8, kind="ExternalInput")
    wvd8_d = nc.dram_tensor("wvd8", [D, 2 * GW], fp8, kind="ExternalInput")
    wv16_d = nc.dram_tensor("wv16", [D, GW], fp16, kind="ExternalInput")
    wo_d = nc.dram_tensor("wo", [GW, D], fp16, kind="ExternalInput")
    # consts packed: stA [:,0:128], stB [:,128:256], sel [0:65,256:384],
    # hole 16s [0:1,384:512], hole -120s [0:1,512:1024]
    consts_d = nc.dram_tensor("consts", [128, 1024], fp16, kind="ExternalInput")
    outT_d = nc.dram_tensor("outT", [D, S], fp16, kind="ExternalOutput")

    KC = D // 128  # 8 fp16 contraction chunks (v16 projection)

    with tile.TileContext(nc) as tc:
        with (
            tc.tile_pool(name="weights", bufs=1) as wpool,
            tc.tile_pool(name="acts", bufs=1) as apool,
            tc.tile_pool(name="psmm", bufs=2, space=PSUM) as psmm,
            tc.tile_pool(name="psav", bufs=2, space=PSUM) as psav,
            tc.tile_pool(name="pflt", bufs=2, space=PSUM) as pflt,
            tc.tile_pool(name="epool", bufs=30) as epool,
            tc.tile_pool(name="norm", bufs=3) as npool,
            tc.tile_pool(name="outp", bufs=8) as opool,
            tc.tile_pool(name="qkst", bufs=6) as qkpool,
            tc.tile_pool(name="bounce", bufs=6, space="DRAM") as bpool,
        ):
            # ---- constants + fp8 weights (ahead of everything) ----
            consts = wpool.tile([128, 1024], fp16, tag="consts")
            stA = consts[:, 0:128]
            stB = consts[:, 128:256]
            sel_sb = consts[0:65, 256:384]
            hole16 = consts[0:1, 384:512]
            holem = consts[0:1, 512:1024]
            wqk_sb = wpool.tile([128, KCP, 2, 2 * GW], fp8, tag="wqk")
            wvd8_sb = wpool.tile([128, KCP, 2, 2 * GW], fp8, tag="wvd8")
            wv16_sb = wpool.tile([128, KC, GW], fp16, tag="wv16")
            wo_sb = wpool.tile([128, 2, D], fp16, tag="wo")
            nc.sync.dma_start(
                wqk_sb, wqk8_d.ap().rearrange("(kcp i p) m -> p kcp i m", p=128, i=2))

            first_rep = True
            for _rep in range(reps):  # >1 only for timing builds
                xT8_sb = apool.tile([128, KCP, 2, S], fp8, tag="xT8",
                                    name=f"xT8_{_rep}")
                xT8_view = xT8_d.ap().rearrange("(kcp i p) s -> p kcp i s",
                                                p=128, i=2)
                # columns 0:512 only — all the prefix needs; rest follows
                # after the prefix bounce DMAs are queued
                nc.sync.dma_start(xT8_sb[:, :, :, 0:QC],
                                  xT8_view[:, :, :, 0:QC])
                if first_rep:
                    nc.sync.dma_start(consts, consts_d.ap())
                    # preload the Exp act table off the critical path
                    tdum = npool.tile([128, 128], fp16, tag="tdum")
                    nc.scalar.activation(tdum, stA, Exp, scale=0.125)

                qT8 = [apool.tile([32, 2, 2, S], fp8, tag=f"qT8_{mi}",
                                  name=f"qT8_{mi}_{_rep}") for mi in range(2)]
                kT8 = [apool.tile([32, 2, 2, S], fp8, tag=f"kT8_{mi}",
                                  name=f"kT8_{mi}_{_rep}") for mi in range(2)]
                # q/k for chunks 0..1 in fp16 (feature rows f = 64h + dk):
                # scores for q < 1024 skip the DRAM bounce round-trip;
                # [128, qk, mi, q] so one descale covers q AND k of a pair
                qkT16 = apool.tile([128, 2, 2, 2 * QC], fp16, tag="qkT16")
                v16 = apool.tile([128, 4, HPC * 128], fp16, tag="v16")
                v8 = apool.tile([128, S // 128, HPC * 128], fp8, tag="v8")
                v16p = v16.rearrange("p st (h2 par w) -> p st h2 par w",
                                     par=2, w=128)
                v8p = v8.rearrange("p st (h2 par w) -> p st h2 par w",
                                   par=2, w=128)
                v8dr = v8.rearrange("p (jp i) m -> p jp i m", i=2)
                attnT = apool.tile([128, 2, S], fp16, tag="attnT")

                def emit_qk_tail(ps, dst, mi, c2, di):
                    # psum -> fp8 sbuf (descale 1/64) -> DRAM -> shuffled
                    st8 = qkpool.tile([128, QC], fp8, tag="qkst")
                    nc.vector.tensor_scalar_mul(st8, ps, 1.0 / 64.0)
                    bt = bpool.tile([128, QC], fp8, tag="bnc",
                                    name=f"bnc{di}_{mi}_{c2}")
                    nc.gpsimd.dma_start(bt, st8)
                    nc.sync.dma_start(
                        dst[:, :, :, QC * c2:QC * (c2 + 1)],
                        bt.rearrange("(h i p) q -> p h i q", p=32, i=2))

                # ---- prefix: q,k chunk-0 projections, chunk-major ----
                pqk = [psmm.tile([128, 2, QC], f32, tag="mm", name=f"pqk{mi}")
                       for mi in range(2)]
                for mi in range(2):
                    for kcp in range(KCP):
                        for di in range(2):
                            for n0 in (0, QC // 2):
                                nc.tensor.matmul(
                                    pqk[mi][:, di, n0:n0 + QC // 2],
                                    lhsT=wqk_sb[:, kcp, :,
                                                GW * di + 128 * mi:
                                                GW * di + 128 * (mi + 1)],
                                    rhs=xT8_sb[:, kcp, :, n0:n0 + QC // 2],
                                    start=(kcp == 0 and n0 == 0),
                                    stop=(kcp == KCP - 1 and n0 != 0),
                                    perf_mode=DR,
                                )
                    with nc.allow_low_precision(reason="fp16 q/k chunk 0"):
                        nc.vector.tensor_scalar_mul(
                            qkT16[:, :, mi, 0:QC], pqk[mi], 1.0 / 64.0)
                for mi in range(2):
                    # kT8 cols 0:512 are consumed by all later chunks
                    emit_qk_tail(pqk[mi][:, 1, :], kT8[mi], mi, 0, 1)
                for vp in (v16p, v8p):
                    nc.gpsimd.memset(vp[:, :, :, 0, DK + 1:128], 0.0)
                    nc.gpsimd.memset(vp[:, :, :, 1, 0:32], 0.0)
                    nc.gpsimd.memset(vp[:, :, :, 1, 33:DK], 0.0)
                    nc.gpsimd.memset(vp[:, :, :, 0, DK:DK + 1], 1.0)
                    nc.gpsimd.memset(vp[:, :, :, 1, 32:33], 1.0)

                # ---- bulk inputs AFTER the prefix bounce DMAs ----
                # order: v16 deps (wv16, xT16s) so chunk-0 AV unblocks ~8us,
                # then xT8 cols 512:1024 (q/k chunk-1 projections), the rest
                # of xT8, v8 weights, wo.
                xT16_sb = apool.tile([128, KC, QC], fp16, tag="xT16",
                                     name=f"xT16_{_rep}")
                xT16_view = xT16_d.ap().rearrange("(kc p) s -> p kc s", p=128)
                nc.sync.dma_start(xT8_sb[:, :, :, QC:2 * QC],
                                  xT8_view[:, :, :, QC:2 * QC])
                if first_rep:
                    first_rep = False
                    nc.sync.dma_start(
                        wv16_sb,
                        wv16_d.ap().rearrange("(kc p) m -> p kc m", p=128))
                nc.sync.dma_start(xT16_sb[:, :, 0:QC // 2],
                                  xT16_view[:, :, 0:QC // 2])
                nc.sync.dma_start(xT16_sb[:, :, QC // 2:],
                                  xT16_view[:, :, QC // 2:])
                if _rep == 0:
                    nc.sync.dma_start(
                        wvd8_sb,
                        wvd8_d.ap().rearrange("(kcp i p) m -> p kcp i m",
                                              p=128, i=2))
                nc.sync.dma_start(xT8_sb[:, :, :, 2 * QC:],
                                  xT8_view[:, :, :, 2 * QC:])
                if _rep == 0:
                    nc.sync.dma_start(
                        wo_sb, wo_d.ap().rearrange("(f p) n -> p f n", p=128))

                def wo_unit(c2, dm, po=None, act_ob=False):
                    if po is None:
                        po = pflt.tile([128, QC], f32, tag="flt")
                    for f in range(2):
                        nc.tensor.matmul(
                            po,
                            lhsT=wo_sb[:, f, 128 * dm:128 * (dm + 1)],
                            rhs=attnT[:, f, QC * c2:QC * (c2 + 1)],
                            start=(f == 0),
                            stop=(f == 1),
                        )
                    ob = opool.tile([128, QC], fp16, tag="ob")
                    if act_ob:
                        nc.scalar.copy(ob, po)
                    else:
                        nc.vector.tensor_copy(ob, po)
                    nc.sync.dma_start(
                        outT_d.ap()[128 * dm:128 * (dm + 1),
                                    QC * c2:QC * (c2 + 1)],
                        ob,
                    )

                # ---- PE filler machinery ----
                from collections import deque
                fq = deque()
                gens = {}

                def push(key, g):
                    gens[key] = g
                    fq.append(key)

                def fill(n):
                    done = 0
                    while done < n and fq:
                        k = fq[0]
                        g = gens.get(k)
                        if g is None:
                            fq.popleft()
                            continue
                        try:
                            next(g)
                            done += 1
                        except StopIteration:
                            del gens[k]
                            fq.popleft()

                def need(*keys):
                    for k in keys:
                        g = gens.pop(k, None)
                        if g is None:
                            continue
                        for _ in g:
                            pass

                def flush():
                    while fq or gens:
                        if not fq:
                            need(*list(gens))
                            break
                        fill(64)

                def g_qk(di, mi, c2):
                    dst = (qT8, kT8)[di]
                    ps = pflt.tile([128, QC], f32, tag="flt")
                    q0 = QC * c2
                    for kcp in range(KCP):
                        for n0 in (0, QC // 2):
                            nc.tensor.matmul(
                                ps[:, n0:n0 + QC // 2],
                                lhsT=wqk_sb[:, kcp, :,
                                            GW * di + 128 * mi:
                                            GW * di + 128 * (mi + 1)],
                                rhs=xT8_sb[:, kcp, :,
                                           q0 + n0:q0 + n0 + QC // 2],
                                start=(kcp == 0 and n0 == 0),
                                stop=(kcp == KCP - 1 and n0 != 0),
                                perf_mode=DR,
                            )
                        if kcp == KCP - 1:
                            emit_qk_tail(ps, dst[mi], mi, c2, di)
                        yield

                def g_qk16(mi):
                    # chunk-1 q/k: fp16 into qkT16 (no bounce on the scores
                    # path); k additionally descaled to fp8 and bounced into
                    # kT8 for chunks >= 2.
                    q0 = QC
                    for di in range(2):
                        ps = pflt.tile([128, QC], f32, tag="flt",
                                       name=f"qk16_{di}_{mi}")
                        for kcp in range(KCP):
                            for n0 in (0, QC // 2):
                                nc.tensor.matmul(
                                    ps[:, n0:n0 + QC // 2],
                                    lhsT=wqk_sb[:, kcp, :,
                                                GW * di + 128 * mi:
                                                GW * di + 128 * (mi + 1)],
                                    rhs=xT8_sb[:, kcp, :,
                                               q0 + n0:q0 + n0 + QC // 2],
                                    start=(kcp == 0 and n0 == 0),
                                    stop=(kcp == KCP - 1 and n0 != 0),
                                    perf_mode=DR,
                                )
                            if kcp == KCP - 1:
                                with nc.allow_low_precision(
                                        reason="fp16 q/k chunk 1"):
                                    nc.vector.tensor_scalar_mul(
                                        qkT16[:, di, mi, QC:2 * QC], ps,
                                        1.0 / 64.0)
                                if di == 1:
                                    emit_qk_tail(ps, kT8[mi], mi, 1, 1)
                            yield

                def g_v8(st):
                    # v8 = x8 @ (wv8 + dwv8), both x64; descale 1/64 in copy
                    ps = pflt.tile([128, QC], f32, tag="flt")
                    n = 0
                    for kcp in range(KCP):
                        for half in range(2):
                            nc.tensor.matmul(
                                ps[:, 0:GW],
                                lhsT=xT8_sb[:, kcp, :, 128 * st:128 * (st + 1)],
                                rhs=wvd8_sb[:, kcp, :,
                                            GW * half:GW * (half + 1)],
                                start=(n == 0),
                                stop=(n == 2 * KCP - 1),
                                perf_mode=DR,
                            )
                            n += 1
                        if kcp == KCP - 1:
                            pv = ps[:, 0:GW].rearrange(
                                "p (h2 par w) -> p h2 par w", par=2, w=DK)
                            with nc.allow_low_precision(reason="fp8 v tile"):
                                nc.vector.tensor_scalar_mul(
                                    v8p[:, st, :, 0, 0:DK], pv[:, :, 0, :],
                                    1.0 / 64.0)
                                nc.vector.tensor_scalar_mul(
                                    v8p[:, st, :, 1, DK:2 * DK], pv[:, :, 1, :],
                                    1.0 / 64.0)
                        yield

                def g_v16(st):
                    ps = pflt.tile([128, QC], f32, tag="flt")
                    for kc in range(KC):
                        nc.tensor.matmul(
                            ps[:, 0:GW],
                            lhsT=xT16_sb[:, kc, 128 * st:128 * (st + 1)],
                            rhs=wv16_sb[:, kc, :],
                            start=(kc == 0),
                            stop=(kc == KC - 1),
                        )
                        if kc == KC - 1:
                            pv = ps[:, 0:GW].rearrange(
                                "p (h2 par w) -> p h2 par w", par=2, w=DK)
                            nc.vector.tensor_copy(v16p[:, st, :, 0, 0:DK],
                                                  pv[:, :, 0, :])
                            nc.vector.tensor_copy(v16p[:, st, :, 1, DK:2 * DK],
                                                  pv[:, :, 1, :])
                        yield

                def g_wo(c2, dm, act_ob=False):
                    po = pflt.tile([128, QC], f32, tag="flt")
                    for f in range(2):
                        nc.tensor.matmul(
                            po,
                            lhsT=wo_sb[:, f, 128 * dm:128 * (dm + 1)],
                            rhs=attnT[:, f, QC * c2:QC * (c2 + 1)],
                            start=(f == 0),
                            stop=(f == 1),
                        )
                        if f == 1:
                            ob = opool.tile([128, QC], fp16, tag="ob")
                            if act_ob:
                                nc.scalar.copy(ob, po)
                            else:
                                nc.vector.tensor_copy(ob, po)
                            nc.sync.dma_start(
                                outT_d.ap()[128 * dm:128 * (dm + 1),
                                            QC * c2:QC * (c2 + 1)],
                                ob,
                            )
                        yield

                def g_wo_half(qb, w, dm, act_ob=False):
                    po = pflt.tile([128, QC], f32, tag="flt")
                    for f in range(2):
                        nc.tensor.matmul(
                            po[:, 0:w],
                            lhsT=wo_sb[:, f, 128 * dm:128 * (dm + 1)],
                            rhs=attnT[:, f, qb:qb + w],
                            start=(f == 0),
                            stop=(f == 1),
                        )
                        if f == 1:
                            ob = opool.tile([128, QC], fp16, tag="ob")
                            if act_ob:
                                nc.scalar.copy(ob[:, 0:w], po[:, 0:w])
                            else:
                                nc.vector.tensor_copy(ob[:, 0:w], po[:, 0:w])
                            nc.sync.dma_start(
                                outT_d.ap()[128 * dm:128 * (dm + 1),
                                            qb:qb + w],
                                ob[:, 0:w],
                            )
                        yield

                def wo_half_unit(qb, w, dm, po=None, act_ob=False):
                    if po is None:
                        pot = pflt.tile([128, QC], f32, tag="flt",
                                        name=f"woh_{dm}")
                        po = pot[:, 0:w]
                    for f in range(2):
                        nc.tensor.matmul(
                            po,
                            lhsT=wo_sb[:, f, 128 * dm:128 * (dm + 1)],
                            rhs=attnT[:, f, qb:qb + w],
                            start=(f == 0),
                            stop=(f == 1),
                        )
                    ob = opool.tile([128, QC], fp16, tag="ob")
                    if act_ob:
                        nc.scalar.copy(ob[:, 0:w], po)
                    else:
                        nc.vector.tensor_copy(ob[:, 0:w], po)
                    nc.sync.dma_start(
                        outT_d.ap()[128 * dm:128 * (dm + 1), qb:qb + w],
                        ob[:, 0:w],
                    )

                class AttUnit:
                    """Attention for pair mi over q-window [qb, qb+w).

                    sc(hh, t) emits scores psum + exp for jpair t of head
                    hh; run() does AV + lookahead, weaves the next unit's
                    first scores in before normalize(). need_fn(t) lazily
                    drains the v-tile generators jpair t consumes.
                    """

                    def __init__(self, mi, qb, w):
                        self.mi, self.qb, self.w = mi, qb, w
                        self.ntp = (qb + w) // 256
                        self.fp8_av = qb >= QC
                        self.fp16_sc = qb + w <= 2 * QC
                        self.es = {}
                        self.avs = None

                    def sc(self, hh, t):
                        mi, qb, w = self.mi, self.qb, self.w
                        j0 = 2 * t
                        k0 = KT * j0
                        lo0 = min(max(qb, k0), qb + w)
                        lo1 = min(max(qb, k0 + KT), qb + w)
                        of0, of1 = lo0 - qb, lo1 - qb
                        ps = psmm.tile([128, 2, w], f32, tag="mm")
                        for jj, (lo, of) in ((0, (lo0, of0)), (1, (lo1, of1))):
                            kk = k0 + KT * jj
                            diag = kk >= qb
                            if self.fp16_sc:
                                nc.tensor.matmul(
                                    ps[:, jj, of:w],
                                    lhsT=qkT16[64 * hh:64 * (hh + 1), 1, mi,
                                               kk:kk + KT],
                                    rhs=qkT16[64 * hh:64 * (hh + 1), 0, mi,
                                              lo:qb + w],
                                    start=True,
                                    stop=not diag,
                                )
                            else:
                                chunks = []
                                n0 = of
                                while n0 < w:
                                    nn = min(w - n0, QC // 2)
                                    chunks.append((n0, nn))
                                    n0 += nn
                                for ci, (n0, nn) in enumerate(chunks):
                                    nc.tensor.matmul(
                                        ps[:, jj, n0:n0 + nn],
                                        lhsT=kT8[mi][:, hh, :, kk:kk + KT],
                                        rhs=qT8[mi][:, hh, :,
                                                    qb + n0:qb + n0 + nn],
                                        start=(ci == 0),
                                        stop=(not diag and
                                              ci == len(chunks) - 1),
                                        perf_mode=DR,
                                    )
                            if jj == 1 and of1 > of0:
                                nc.tensor.matmul(
                                    ps[:, 1, of0:of1],
                                    lhsT=hole16,
                                    rhs=holem[0:1, 0:of1 - of0],
                                    start=False,
                                    stop=False,
                                )
                            if diag:
                                nc.tensor.matmul(
                                    ps[:, jj, of:of + KT],
                                    lhsT=stA,
                                    rhs=stB,
                                    start=False,
                                    stop=True,
                                )
                        e = epool.tile([128, 2, w],
                                       fp8 if self.fp8_av else fp16, tag="e")
                        nc.scalar.activation(
                            e[:, :, of0:w], ps[:, :, of0:w], Exp, scale=0.125
                        )
                        self.es[(hh, t)] = (e, of0, of1)

                    def av(self, hh, t):
                        e, of0, of1 = self.es.pop((hh, t))
                        h = 2 * self.mi + hh
                        ntp, w = self.ntp, self.w
                        if self.fp8_av:
                            n0 = of0
                            while n0 < w:
                                nn = min(w - n0, QC // 2)
                                nc.tensor.matmul(
                                    self.avs[hh][:, n0:n0 + nn],
                                    lhsT=v8dr[:, t, :, h * 128:(h + 1) * 128],
                                    rhs=e[:, :, n0:n0 + nn],
                                    start=(t == 0 and n0 == of0),
                                    stop=(t == ntp - 1 and n0 + nn >= w),
                                    perf_mode=DR,
                                )
                                n0 += nn
                        else:
                            j0 = 2 * t
                            for jj, of in ((0, of0), (1, of1)):
                                nc.tensor.matmul(
                                    self.avs[hh][:, of:w],
                                    lhsT=v16[:, j0 + jj,
                                             h * 128:(h + 1) * 128],
                                    rhs=e[:, jj, of:w],
                                    start=(t == 0 and jj == 0),
                                    stop=(t == ntp - 1 and jj == 1),
                                )

                    def run(self, nxt=None, need_fn=None, fill_n=2):
                        mi, qb, w, ntp = self.mi, self.qb, self.w, self.ntp
                        self.avs = (
                            psav.tile([128, w], f32, tag="av", name="av_e"),
                            psav.tile([128, w], f32, tag="av", name="av_o"),
                        )
                        for t in range(ntp):
                            if t + 1 < ntp:
                                self.sc(0, t + 1)
                                self.sc(1, t + 1)
                            if need_fn is not None:
                                need_fn(t)
                            self.av(0, t)
                            fill(fill_n)
                            self.av(1, t)
                            fill(fill_n)
                        fill(4)
                        if nxt is not None:
                            nxt.sc(0, 0)
                            nxt.sc(1, 0)
                        av_e, av_o = self.avs
                        rden = npool.tile([DK + 1, w], fp16, tag="rden")
                        with nc.allow_low_precision(reason="fp16 1/den"):
                            nc.vector.reciprocal(rden[DK:DK + 1, :],
                                                 av_e[DK:DK + 1, :])
                            nc.vector.reciprocal(rden[32:33, :],
                                                 av_o[32:33, :])
                        bcps = psmm.tile([128, 2, w], f32, tag="mm")
                        bc = bcps[:, 0, :]
                        nc.tensor.matmul(
                            bc[0:DK, :],
                            lhsT=sel_sb[DK:DK + 1, 0:DK],
                            rhs=rden[DK:DK + 1, :],
                            start=True,
                            stop=True,
                        )
                        nc.tensor.matmul(
                            bc[DK:128, :],
                            lhsT=sel_sb[32:33, 0:DK],
                            rhs=rden[32:33, :],
                            start=True,
                            stop=True,
                        )
                        bc_sb = npool.tile([128, w], f32, tag="bcsb")
                        nc.vector.tensor_copy(bc_sb, bc)
                        nc.vector.tensor_mul(
                            attnT[0:DK, mi, qb:qb + w], av_e[0:DK, :],
                            bc_sb[0:DK, :]
                        )
                        nc.vector.tensor_mul(
                            attnT[DK:128, mi, qb:qb + w], av_o[DK:128, :],
                            bc_sb[DK:128, :]
                        )

                # ---- schedule ----
                # (0,0) first (fp16 path, no bounce); (1,0) LAST, split in
                # two 256-wide halves so the final wo chunk is half-hidden
                # under the second half's exps.
                def push_qk(mi, c2):
                    push(("q", mi, c2), g_qk(0, mi, c2))
                    push(("k", mi, c2), g_qk(1, mi, c2))

                def vneed16(t):
                    need(("v16", 2 * t), ("v16", 2 * t + 1))

                def vneed8(t):
                    need(("v8", 2 * t), ("v8", 2 * t + 1))

                u00 = AttUnit(0, 0, QC)
                u10 = AttUnit(1, 0, QC)
                u01 = AttUnit(0, QC, QC)
                u11 = AttUnit(1, QC, QC)
                u02 = AttUnit(0, 2 * QC, QC)
                u12 = AttUnit(1, 2 * QC, QC)
                u03 = AttUnit(0, 3 * QC, QC)
                u13 = AttUnit(1, 3 * QC, QC)

                u00.sc(0, 0)
                u00.sc(1, 0)
                push(("qk16", 0), g_qk16(0))
                push(("qk16", 1), g_qk16(1))
                for st in range(4):
                    push(("v16", st), g_v16(st))
                    push(("v8", st), g_v8(st))
                for st in range(4, 8):
                    push(("v8", st), g_v8(st))

                def u00_need(t):
                    if t == 0:
                        need(("qk16", 0))
                    vneed16(t)

                u00.run(nxt=u10, need_fn=u00_need)
                need(("qk16", 1))
                u10.run(nxt=u01, need_fn=vneed16)
                for dm in range(8):
                    push(("wo", 0, dm), g_wo(0, dm))
                push_qk(0, 2)
                for st in range(8, 12):
                    push(("v8", st), g_v8(st))
                u01.run(nxt=u11, need_fn=vneed8)
                push_qk(1, 2)
                need(("q", 0, 2), ("k", 0, 2))
                u11.run(nxt=u02, need_fn=vneed8)
                for dm in range(8):
                    push(("wo", 1, dm), g_wo(1, dm))
                need(("q", 1, 2), ("k", 1, 2))
                push_qk(0, 3)
                for st in range(12, 16):
                    push(("v8", st), g_v8(st))
                u02.run(nxt=u12, need_fn=vneed8)
                push_qk(1, 3)
                need(("q", 0, 3), ("k", 0, 3))
                u12.run(nxt=u03, need_fn=vneed8)
                for dm in range(8):
                    push(("wo", 2, dm), g_wo(2, dm))
                need(("q", 1, 3), ("k", 1, 3))
                u03.run(nxt=u13, need_fn=vneed8)
                u13.run(nxt=None, need_fn=vneed8)
                flush()

                # tail: wo chunk 3 across all free psum banks. Emit all
                # f=0 matmuls back-to-back first (continuous PE stream
                # ramps the pstate to full clock), then f=1 + ob copies
                # (alternating DVE / idle Act) pipelined behind.
                po2 = psmm.tile([128, 2, QC], f32, tag="mm")
                po3 = psmm.tile([128, 2, QC], f32, tag="mm")
                po4 = psav.tile([128, QC], f32, tag="av", name="wo3_4")
                po5 = psav.tile([128, QC], f32, tag="av", name="wo3_5")
                po6 = pflt.tile([128, QC], f32, tag="flt", name="wo3_6")
                po7 = pflt.tile([128, QC], f32, tag="flt", name="wo3_7")
                pos = [po2[:, 0, :], po2[:, 1, :], po3[:, 0, :],
                       po3[:, 1, :], po4, po5, po6, po7]
                for dm in range(8):
                    nc.tensor.matmul(
                        pos[dm],
                        lhsT=wo_sb[:, 0, 128 * dm:128 * (dm + 1)],
                        rhs=attnT[:, 0, 3 * QC:4 * QC],
                        start=True,
                        stop=False,
                    )
                for dm in range(8):
                    nc.tensor.matmul(
                        pos[dm],
                        lhsT=wo_sb[:, 1, 128 * dm:128 * (dm + 1)],
                        rhs=attnT[:, 1, 3 * QC:4 * QC],
                        start=False,
                        stop=True,
                    )
                    ob = opool.tile([128, QC], fp16, tag="ob",
                                    name=f"ob3_{dm}")
                    if dm % 2 == 1:
                        nc.scalar.copy(ob, pos[dm])
                        nc.gpsimd.dma_start(
                            outT_d.ap()[128 * dm:128 * (dm + 1),
                                        3 * QC:4 * QC],
                            ob,
                        )
                    else:
                        nc.vector.tensor_copy(ob, pos[dm])
                        nc.sync.dma_start(
                            outT_d.ap()[128 * dm:128 * (dm + 1),
                                        3 * QC:4 * QC],
                            ob,
                        )

    nc.compile()
    return nc


def _get_nc():
    if "nc" not in _CACHE:
        _CACHE["nc"] = _build_nc()
    return _CACHE["nc"]


def _consts():
    t = np.arange(128)
    c = np.zeros((128, 1024), np.float16)
    c[:, 0:128] = (t[:, None] <= t[None, :]).astype(np.float16)       # stA
    c[:, 128:256] = np.where(t[:, None] > t[None, :],
                             MASK_STEP, 0.0).astype(np.float16)        # stB
    c[64, 256 + 0:256 + 64] = 1.0                                      # sel
    c[32, 256 + 0:256 + 64] = 1.0
    c[0, 384:512] = 16.0                                               # hole16
    c[0, 512:1024] = -120.0                                            # holem
    return c


def _make_in_maps(x, wq, wk, wv, wo):
    import ml_dtypes
    E4 = ml_dtypes.float8_e4m3

    consts = _consts()
    x = np.asarray(x, np.float32)
    x16s = [np.ascontiguousarray(x[b].T).astype(np.float16) for b in range(B)]
    x8s = [xt.astype(E4) for xt in x16s]
    x16small = [np.ascontiguousarray(xt[:, 0:QC]) for xt in x16s]
    wq8 = (np.asarray(wq, np.float32) * 64.0).astype(E4)
    wk8 = (np.asarray(wk, np.float32) * 64.0).astype(E4)
    wv64 = np.asarray(wv, np.float32) * 64.0
    wv8 = wv64.astype(E4)
    dwv8 = (wv64 - wv8.astype(np.float32)).astype(E4)
    wv16 = np.asarray(wv, np.float32).astype(np.float16)
    woh = np.asarray(wo, np.float32).astype(np.float16)
    in_maps = []
    for c in range(NCORES):
        b, g = divmod(c, HPC)
        cols = slice(g * GW, (g + 1) * GW)
        in_maps.append({
            "xT8": x8s[b],
            "xT16": x16small[b],
            "wqk8": np.ascontiguousarray(
                np.concatenate([wq8[:, cols], wk8[:, cols]], axis=1)),
            "wvd8": np.ascontiguousarray(
                np.concatenate([wv8[:, cols], dwv8[:, cols]], axis=1)),
            "wv16": np.ascontiguousarray(wv16[:, cols]),
            "wo": np.ascontiguousarray(woh[cols, :]),
            "consts": consts,
        })
    return in_maps


def run(x, wq, wk, wv, wo, trace=False):
    from concourse.bass_utils import run_bass_kernel_spmd

    nc = _get_nc()
    in_maps = _make_in_maps(x, wq, wk, wv, wo)
    res = run_bass_kernel_spmd(nc, in_maps, list(range(NCORES)), trace=trace)
    acc = np.zeros((B, D, S), np.float64)
    for c in range(NCORES):
        acc[c // HPC] += res.results[c]["outT"]
    out = np.ascontiguousarray(acc.transpose(0, 2, 1).astype(np.float32))
    return out, res


def kernel(x, wq, wk, wv, wo):
    out, _ = run(x, wq, wk, wv, wo, trace=False)
    return out
